# revision 42
# baseline (speedup 1.0000x reference)
"""Trainium2 Bass kernel for nn_AndLayer (permutation-based AND layer).

Math (see reference):
    tk = tanh(kernel)                 # [448, C=128]
    q  = 1 - tk^2
    For each batch b and permutation k=(o0,o1) of 8 objects (K=56 perms):
        in_vec[448] = [nullary(64) | unary[o0](128) | unary[o1](128)
                       | binary[o0,o1'](64) | binary[o1,o0'](64)]
        conj[b,k,c] = min_i (in_vec[i]*tk[i,c] + q[i,c])
        out[b,c]    = max_k conj[b,k,c]

Decomposition (exact):
    nmin[b,c]     = min over nullary 64 rows        (shared by all k)
    umin_v[b,o,c] = min over unary   128 rows       (16 combos per b)
    bmin[b,k,c]   = min over binary 128 rows        (per k)
    conj = min(nmin, umin0[o0], umin1[o1], bmin[k]); out = max_k conj

Device strategy (per core, data-parallel over B: 4 batches/core):
    One matmul per 64-pred half-tile computes in*tk + q directly: the
    stationary stacks [tk_half ; 1-tk_half^2] (K=128) and the moving
    operand stacks [diag(in_half) ; I64].  Output lands in PSUM
    transposed ([c, pred]) so the min-reduce is a free-axis reduce.

    v3 notes:
    - Binary atlas deduplicated: diag(binf[b,k]) serves as the A-block
      of perm k and the B-block of rev(k).  Blocks stored once in
      (pair, b, dir) order; B-matmuls read them with a dir-flipped
      (negative-stride) access pattern.  3.67MB instead of 7.34MB.
    - PSUM drains: only Vector can min and only Vector/Scalar can read
      PSUM (GpSimd has no PSUM port, no pairwise ops).  W-class waves:
      Scalar copies PSUM->SBUF bf16, Vector min-folds at 2x, batched in
      wave pairs; Z-class waves: one Vector tensor_reduce from PSUM.
      Z-waves placed at the end (shortest tail) and where Scalar is
      busy.  GpSimd does the bmin->grid scatter and q-prep.
    - DMA triggers issued in need-order; kern split so tanh starts as
      early as possible; dummy activation preloads the tanh table.
"""

import itertools
import os
import sys

import numpy as np

for _p in ("/opt/trn_rl_repo", "/root/.axon_site/_ro/trn_rl_repo"):
    if os.path.isdir(_p) and _p not in sys.path:
        sys.path.insert(0, _p)

import concourse.bass as bass  # noqa: E402
import concourse.bacc as bacc  # noqa: E402
import concourse.mybir as mybir  # noqa: E402
import concourse.tile as tile  # noqa: E402
from concourse.bass import AP  # noqa: E402
from concourse.bass_utils import run_bass_kernel_spmd  # noqa: E402

import ml_dtypes  # noqa: E402

BF16 = ml_dtypes.bfloat16

# Problem constants (hardcoded per spec)
B, N, V = 32, 8, 2
P0, P1, P2, C = 64, 128, 64, 128
K = 56  # permutations of 2 from 8
NPAIR = 28
NCORES = 8
BL = B // NCORES  # 4 batches per core
NBT = BL * K  # binary tiles per core = 224
NBW = 14  # binary waves of 16 tiles (2 pairs x 4 b x 2 d)

F32 = mybir.dt.float32
BF16_T = mybir.dt.bfloat16
MIN_OP = mybir.AluOpType.min

# chunk order in the rearranged kernel tensor: unary (1..4), binary (5,6),
# nullary (0) last.  CHUNK[ci] = original 64-row chunk index.
CHUNK = [1, 2, 3, 4, 5, 6, 0]


def _pair_tables():
    perm_idxs = np.array(list(itertools.permutations(range(N), V)))  # [56, 2]
    k_of = {tuple(p): i for i, p in enumerate(perm_idxs)}
    pairs = [(i, j) for i in range(N) for j in range(i + 1, N)]  # lex order
    kidx = np.array([[k_of[(i, j)], k_of[(j, i)]] for (i, j) in pairs])
    pstart = np.array([7 * i - i * (i - 1) // 2 for i in range(N)])  # P(i)
    return pairs, kidx, pstart


PAIRS, KIDX, PSTART = _pair_tables()


def build_graph():
    nc = bacc.Bacc("TRN2", debug=False)

    kern_d = nc.declare_dram_parameter("kern", [128, 7 * 128], F32, isOutput=False)
    aun_d = nc.declare_dram_parameter("aun", [128, BL * N * 128 + BL * 64], BF16_T, isOutput=False)
    abin_d = nc.declare_dram_parameter("abin", [128, NBT * 64], BF16_T, isOutput=False)
    out_d = nc.declare_dram_parameter("out", [128, BL], F32, isOutput=True)

    with tile.TileContext(nc) as tc:
        with (
            tc.tile_pool(name="const", bufs=1) as const,
            tc.tile_pool(name="work", bufs=2) as work,
            tc.tile_pool(name="drain", bufs=2) as dr,
            tc.tile_pool(name="psum", bufs=2, space="PSUM") as psum_pool,
        ):
            # ---- tanh activation-table preload (overlaps the input DMAs) ----
            dum = const.tile([128, 512], BF16_T, tag="dum")
            dout = const.tile([128, 8], BF16_T, tag="dout")
            nc.gpsimd.memset(dum[:], 0.0)
            nc.scalar.activation(
                dout[:], dum[:, 0:8], mybir.ActivationFunctionType.Tanh
            )

            # ---- input DMAs, in need-order on the Sync queue ----
            raw = const.tile([128, 896], F32, tag="raw")
            aun_s = const.tile([128, BL * N * 128 + BL * 64], BF16_T, tag="aun")
            nc.sync.dma_start(raw[:, 0:256], kern_d[:, 0:256])
            nc.sync.dma_start(aun_s[:, 0:1024], aun_d[:, 0:1024])
            nc.sync.dma_start(raw[:, 256:512], kern_d[:, 256:512])
            nc.sync.dma_start(aun_s[:, 1024:2048], aun_d[:, 1024:2048])
            nc.sync.dma_start(raw[:, 512:896], kern_d[:, 512:896])
            nc.sync.dma_start(aun_s[:, 2048:4352], aun_d[:, 2048:4352])
            abin_s = const.tile([128, NBT * 64], BF16_T, tag="abin")
            for ch in range(5):  # first 5 chunks on Sync
                nc.sync.dma_start(
                    abin_s[:, ch * 2048 : (ch + 1) * 2048],
                    abin_d[:, ch * 2048 : (ch + 1) * 2048],
                )

            # ---- PE P-state warmup: dummy matmuls on zeros while DMAs run.
            # The PE clock ramps 1.2 -> 2.4 GHz only under sustained matmul
            # activity; warm it so the real waves run at full speed. ----
            wps = psum_pool.tile([128, 512], F32, tag="ps")
            for _ in range(11):
                nc.tensor.matmul(
                    wps[:], dum[:, 0:128], dum[:], start=True, stop=True
                )

            # ---- stationaries: st chunk ci = [tanh(rows); 1-tanh^2].
            # Emitted in pieces, interleaved with the first waves (see
            # schedule) so the Scalar queue stays dense. ----
            st = const.tile([128, 896], BF16_T, tag="st")
            sq = work.tile([64, 896], F32, tag="sq")

            def prep(lo, hi, on_v=False):
                nc.scalar.activation(
                    st[:, lo:hi], raw[:, lo:hi], mybir.ActivationFunctionType.Tanh
                )
                # square + fused 1 - tk^2 (cast to bf16 into the q half).
                # Early pieces run on the still-idle vector engine so the
                # scalar queue moves straight to the next tanh; later on
                # scalar+gpsimd since vector paces the drains.
                if on_v:
                    nc.vector.tensor_tensor(
                        sq[:, lo:hi], st[64:128, lo:hi], st[64:128, lo:hi],
                        mybir.AluOpType.mult,
                    )
                    nc.vector.tensor_scalar(
                        st[64:128, lo:hi], sq[:, lo:hi], -1.0, 1.0,
                        mybir.AluOpType.mult, mybir.AluOpType.add,
                    )
                else:
                    nc.scalar.activation(
                        sq[:, lo:hi], st[64:128, lo:hi],
                        mybir.ActivationFunctionType.Square,
                    )
                    nc.gpsimd.tensor_scalar(
                        st[64:128, lo:hi], sq[:, lo:hi], -1.0, 1.0,
                        mybir.AluOpType.mult, mybir.AluOpType.add,
                    )

            for ch in range(5, 7):
                nc.sync.dma_start(
                    abin_s[:, ch * 2048 : (ch + 1) * 2048],
                    abin_d[:, ch * 2048 : (ch + 1) * 2048],
                )
            stc = [st[:, ci * 128 : (ci + 1) * 128] for ci in range(7)]
            st_u0a, st_u0b, st_u1a, st_u1b, st_ba, st_bb, st_n = stc

            # ---- accumulators ----
            bmin = const.tile([128, NBT], BF16_T, tag="bmin")  # (pair, b, d)
            um = const.tile([128, BL * 16], BF16_T, tag="um")  # (b, pos, o)
            nm = const.tile([128, BL], BF16_T, tag="nm")
            grid = const.tile([128, BL * 64], BF16_T, tag="grid")
            gscr = const.tile([128, BL * 64], BF16_T, tag="gscr")
            outf = const.tile([128, BL], F32, tag="outf")

            bmv = bmin[:].rearrange("p (pr b d) -> p pr b d", b=BL, d=2)
            um4 = um[:].rearrange("p (b q o) -> p b q o", b=BL, q=2)
            g4v = grid[:].rearrange("p (b r c) -> p b r c", b=BL, r=8)

            # ---------------- drain helpers ----------------
            def drain_Z(ps, dst):
                nc.vector.tensor_reduce(
                    dst,
                    ps[:].rearrange("p (c i f) -> p i c f", c=2, f=64),
                    mybir.AxisListType.XY,
                    MIN_OP,
                )

            def copy_W(ps, scr_half):
                nc.scalar.activation(
                    scr_half, ps[:], mybir.ActivationFunctionType.Copy
                )

            def folds_W(scr_ap, dsts):
                """scr_ap [128, n*2048] (n waves, each [A:1024|B:1024]);
                dsts: one AP per wave (16 mins each), or a single AP for
                all n*16 mins when contiguous."""
                n = scr_ap.shape[1] // 2048
                w = 1024 * n
                s3 = scr_ap.rearrange("p (w c f) -> p c w f", c=2, f=1024)
                t0 = dr.tile([128, w], BF16_T, tag=f"t0W{n}")
                nc.vector.tensor_tensor(
                    t0[:].rearrange("p (w f) -> p w f", f=1024), s3[:, 0], s3[:, 1],
                    MIN_OP,
                )
                h0 = t0[:].rearrange("p (t c f) -> p c t f", c=2, f=32)
                t1 = dr.tile([128, w // 2], BF16_T, tag=f"t1W{n}")
                nc.vector.tensor_tensor(
                    t1[:].rearrange("p (t f) -> p t f", f=32), h0[:, 0], h0[:, 1],
                    MIN_OP,
                )
                if len(dsts) == 1:
                    nc.vector.tensor_reduce(
                        dsts[0],
                        t1[:].rearrange("p (t f) -> p t f", f=32),
                        mybir.AxisListType.X,
                        MIN_OP,
                    )
                else:
                    t13 = t1[:].rearrange("p (w t f) -> p w t f", w=n, f=32)
                    for i, dst in enumerate(dsts):
                        nc.vector.tensor_reduce(
                            dst, t13[:, i], mybir.AxisListType.X, MIN_OP
                        )

            # ---------------- wave emitters ----------------
            def unary_mms(g, pos):
                """16 (b,o) tiles for batches {2g, 2g+1}, position pos.
                Atlas per g: [A-blocks x16 | B-blocks x16], contiguous."""
                ps = psum_pool.tile([128, 2048], F32, tag="ps")
                sa = stc[2 * pos]
                sb = stc[2 * pos + 1]
                base = 2048 * g
                for h in range(2):
                    nc.tensor.matmul(
                        ps[:, 512 * h : 512 * h + 512], sa,
                        aun_s[:, base + 512 * h : base + 512 * h + 512],
                        start=True, stop=True,
                    )
                for h in range(2):
                    nc.tensor.matmul(
                        ps[:, 1024 + 512 * h : 1536 + 512 * h], sb,
                        aun_s[:, base + 1024 + 512 * h : base + 1536 + 512 * h],
                        start=True, stop=True,
                    )
                return ps

            def binary_mms(w):
                ps = psum_pool.tile([128, 2048], F32, tag="ps")
                for pl in range(2):  # pair 2w + pl, A-blocks
                    base = 1024 * w + 512 * pl
                    a_ap = AP(
                        tensor=abin_s[:].tensor,
                        offset=abin_s[:].offset + base,
                        ap=[abin_s[:].ap[0], [1, 512]],
                    )
                    nc.tensor.matmul(
                        ps[:, 512 * pl : 512 * pl + 512], st_ba, a_ap,
                        start=True, stop=True,
                    )
                for pl in range(2):  # B-blocks: dir-flipped read (d: -64)
                    base = 1024 * w + 512 * pl
                    b_ap = AP(
                        tensor=abin_s[:].tensor,
                        offset=abin_s[:].offset + base + 64,
                        ap=[abin_s[:].ap[0], [128, 4], [-64, 2], [1, 64]],
                    )
                    nc.tensor.matmul(
                        ps[:, 1024 + 512 * pl : 1536 + 512 * pl], st_bb, b_ap,
                        start=True, stop=True,
                    )
                return ps

            def u_dst(g, pos):
                return um4[:, 2 * g : 2 * g + 2, pos, :]

            def W_solo(mk, dst):
                ps = mk()
                scr = dr.tile([128, 2048], BF16_T, tag="scrS")
                copy_W(ps, scr[:])
                folds_W(scr[:], [dst])

            def Z_split(mk, dst):
                """Per-half tensor_reduce: the A-half TR runs while the PE is
                still writing the B banks; after the last matmul only the
                B-half TR + a tiny combine remain."""
                ps = mk()
                halves = dr.tile([128, 32], BF16_T, tag="zh")
                h3 = halves[:].rearrange("p (c t) -> p c t", c=2)
                for h in range(2):
                    nc.vector.tensor_reduce(
                        h3[:, h],
                        ps[:, 1024 * h : 1024 * h + 1024].rearrange(
                            "p (i f) -> p i f", f=64
                        ),
                        mybir.AxisListType.X,
                        MIN_OP,
                    )
                nc.vector.tensor_tensor(dst, h3[:, 0], h3[:, 1], MIN_OP)

            def scatter_row(x, eng=None):
                """Grid row x (perms (x, j), j > x) and col x (perms (j, x))."""
                eng = eng or nc.gpsimd
                p0, n = PSTART[x], 7 - x
                src0 = bmv[:, p0 : p0 + n, :, 0].transpose([0, 2, 1])
                eng.tensor_copy(g4v[:, :, x, x + 1 : 8], src0)
                src1 = bmv[:, p0 : p0 + n, :, 1].transpose([0, 2, 1])
                eng.tensor_copy(g4v[:, :, x + 1 : 8, x], src1)

            def nullary_wave():
                ps = psum_pool.tile([128, 256], F32, tag="ps")
                nc.tensor.matmul(
                    ps[:], st_n, aun_s[:, BL * N * 128 :], start=True, stop=True
                )
                nc.vector.tensor_reduce(
                    nm[:],
                    ps[:].rearrange("p (t f) -> p t f", f=64),
                    mybir.AxisListType.X,
                    MIN_OP,
                )

            # grid diagonal = -inf before any scatter lands
            nc.vector.memset(
                grid[:].rearrange("p (b c) -> p b c", b=BL)[:, :, 0:64:9], -3.0e38
            )

            # ---------------- schedule ----------------
            # First wave split-Z (Vector starts earliest possible), W-solo
            # through the middle (Scalar paced), full-Z where psum pressure
            # peaks, split-Z last (shortest tail).  Stationary prep pieces
            # are interleaved so the Scalar queue never idles.
            prep(0, 256, on_v=True)
            Z_split(lambda: unary_mms(0, 0), u_dst(0, 0))
            prep(256, 512, on_v=True)
            W_solo(lambda: unary_mms(0, 1), u_dst(0, 1))
            W_solo(lambda: unary_mms(1, 0), u_dst(1, 0))
            prep(512, 768)
            W_solo(lambda: unary_mms(1, 1), u_dst(1, 1))
            prep(768, 896)
            nullary_wave()
            W_solo(lambda: binary_mms(0), bmin[:, 0:16])

            # umask[b,i,j] = min(um0[b,i], um1[b,j], nmin[b]): the combine
            # then needs a single tensor_tensor per grid half at the tail
            umask = const.tile([128, BL * 64], BF16_T, tag="umask")
            um4g = umask[:].rearrange("p (b i j) -> p b i j", b=BL, i=8)
            nc.vector.tensor_tensor(
                um4g,
                um4[:, :, 0, :].unsqueeze(3).to_broadcast((128, BL, 8, 8)),
                um4[:, :, 1, :].unsqueeze(2).to_broadcast((128, BL, 8, 8)),
                MIN_OP,
            )
            nc.vector.tensor_tensor(
                um4g, um4g,
                nm[:].unsqueeze(2).unsqueeze(3).to_broadcast((128, BL, 8, 8)),
                MIN_OP,
            )

            W_solo(lambda: binary_mms(1), bmin[:, 16:32])
            Z_split(lambda: binary_mms(2), bmin[:, 32:48])
            W_solo(lambda: binary_mms(3), bmin[:, 48:64])
            scatter_row(0)
            W_solo(lambda: binary_mms(4), bmin[:, 64:80])
            W_solo(lambda: binary_mms(5), bmin[:, 80:96])
            W_solo(lambda: binary_mms(6), bmin[:, 96:112])
            scatter_row(1)
            W_solo(lambda: binary_mms(7), bmin[:, 112:128])
            W_solo(lambda: binary_mms(8), bmin[:, 128:144])
            scatter_row(2)
            W_solo(lambda: binary_mms(9), bmin[:, 144:160])
            W_solo(lambda: binary_mms(10), bmin[:, 160:176])
            scatter_row(3)

            # combine, first half: grid rows 0-3 are complete once
            # scatter rows 0-3 have landed (their cols only need x<4)
            g4 = grid[:].rearrange("p (b i j) -> p b i j", b=BL, i=8)
            outh = const.tile([128, 2 * BL], F32, tag="outh")
            oh = outh[:].rearrange("p (h b) -> p h b", h=2)
            nc.vector.tensor_tensor(
                g4[:, :, 0:4, :], g4[:, :, 0:4, :], um4g[:, :, 0:4, :], MIN_OP
            )
            nc.vector.tensor_reduce(
                oh[:, 0],
                grid[:].rearrange("p (b h f) -> p b h f", b=BL, h=2)[:, :, 0],
                mybir.AxisListType.X,
                mybir.AluOpType.max,
            )

            W_solo(lambda: binary_mms(11), bmin[:, 176:192])
            W_solo(lambda: binary_mms(12), bmin[:, 192:208])
            scatter_row(4, eng=nc.vector)
            Z_split(lambda: binary_mms(13), bmin[:, 208:224])
            scatter_row(5, eng=nc.vector)
            scatter_row(6, eng=nc.vector)

            # combine, second half + final max
            nc.vector.tensor_tensor(
                g4[:, :, 4:8, :], g4[:, :, 4:8, :], um4g[:, :, 4:8, :], MIN_OP
            )
            nc.vector.tensor_reduce(
                oh[:, 1],
                grid[:].rearrange("p (b h f) -> p b h f", b=BL, h=2)[:, :, 1],
                mybir.AxisListType.X,
                mybir.AluOpType.max,
            )
            nc.vector.tensor_tensor(outf[:], oh[:, 0], oh[:, 1], mybir.AluOpType.max)
            # out-DMA triggered from the (idle) Scalar HWDGE queue: the Sync
            # queue sits in its end-of-kernel wait here and adds latency
            nc.scalar.dma_start(out_d[:], outf[:])

    nc.compile()
    return nc


def _diag_blocks(scales):
    """scales [T, 64] -> [128, T*64] bf16: block t = [diag(scales[t]); I64]."""
    T = scales.shape[0]
    atlas = np.zeros((128, T * 64), dtype=BF16)
    t = np.arange(T)
    j = np.arange(64)
    cols = (t * 64)[:, None] + j[None, :]
    atlas[j[None, :], cols] = scales.astype(BF16)
    atlas[64 + j[None, :], cols] = 1.0
    return atlas


def make_core_inputs(nul, una, binf, ker):
    """Per-core in_map. nul [4,64], una [4,8,128], binf [4,56,64] f32."""
    bl = nul.shape[0]
    # kern: [128, 896] f32, chunk order CHUNK, rows replicated into both halves
    kern = np.empty((128, 896), dtype=np.float32)
    for ci, ch in enumerate(CHUNK):
        rows = ker[64 * ch : 64 * ch + 64]  # [64, 128]
        kern[0:64, ci * 128 : (ci + 1) * 128] = rows
        kern[64:128, ci * 128 : (ci + 1) * 128] = rows
    # binary blocks in (pair, b, d) order: s = binf[b, kidx(pair, d)]
    sc = binf[:, KIDX]  # [b, pair, d, 64]
    sc = sc.transpose(1, 0, 2, 3).reshape(NBT, 64)  # (pair, b, d)
    abin = _diag_blocks(sc)
    # unary: per wave-group g (batches 2g..2g+1): [A-blocks x16 | B-blocks x16]
    su = una.reshape(2, 16, 128)  # [g, (b2 o8), 128]
    su2 = np.concatenate([su[:, :, :64], su[:, :, 64:]], axis=1)  # [g, 32, 64]
    aun_u = _diag_blocks(su2.reshape(64, 64))
    anul = _diag_blocks(nul)  # [128, bl*64]
    aun = np.concatenate([aun_u, anul], axis=1)
    return {
        "kern": kern,
        "aun": np.ascontiguousarray(aun),
        "abin": np.ascontiguousarray(abin),
    }


LAST_RESULTS = None
_GRAPH_CACHE = {}


def get_graph():
    if "nc" not in _GRAPH_CACHE:
        _GRAPH_CACHE["nc"] = build_graph()
    return _GRAPH_CACHE["nc"]


def kernel(nullary_preds, unary_preds, binary_preds, kernel):
    nul = np.asarray(nullary_preds, dtype=np.float32)
    una = np.asarray(unary_preds, dtype=np.float32)
    binf = np.asarray(binary_preds, dtype=np.float32).reshape(B, K, P2)
    ker = np.asarray(kernel, dtype=np.float32)

    nc = get_graph()
    in_maps = []
    for core in range(NCORES):
        bs = slice(core * BL, (core + 1) * BL)
        in_maps.append(make_core_inputs(nul[bs], una[bs], binf[bs], ker))
    res = run_bass_kernel_spmd(nc, in_maps, core_ids=list(range(NCORES)))
    global LAST_RESULTS
    LAST_RESULTS = res
    out = np.concatenate(
        [np.asarray(res.results[i]["out"]).T for i in range(NCORES)], 0
    )
    return out.astype(np.float32)
